# revision 1
# baseline (speedup 1.0000x reference)
"""Trainium2 Bass kernel for nn_MultiHeadCrossAttention (BS=4, S=512, DM=512, H=8).

Sharding: one attention head per NeuronCore (8 heads / 8 cores). Each core
receives the full (transposed) q/k/v plus its head's weight slices, computes
its head end-to-end including the rank-64 slice of the output projection, and
the host sums the 8 partial outputs.

Math restructuring (validated against the reference numerically):
  E^T[kb][j,i] = exp(khT[kb]^T qhT[b]) computed per q-batch b in transposed
  layout; fenmu handled as W = 1/sum_kb E^T (the sqrt(DK)=8 factor is folded
  into Wv/bv on the host); RT = E^T[b] * W; score[i,(c,d)] = RT^T @ vh.
  softmax+LN over d uses: mean(sm) = 1/DK exactly; Sum(sm^2) = Q/Z^2 with
  Z = sum exp(score), Q = sum exp(2*score); std = exp(0.5*ln(63*var) +
  0.5*ln(1/63)); LN sum over c collapses to  sum_c e_c * w1_c  + w0, applied
  via per-partition tensor_scalar (gpsimd) and PE matmul accumulations.
"""

import numpy as np

BS, S, DM, H, DK = 4, 512, 512, 8, 64
EPS = 1e-6
NCORES = 8

F32 = None  # set lazily (mybir import)


def build_program(nc, tile, mybir):
    f32 = mybir.dt.float32
    bf16 = mybir.dt.bfloat16
    i32 = mybir.dt.int32
    AF = mybir.ActivationFunctionType
    OP = mybir.AluOpType
    AX = mybir.AxisListType

    # ---- DRAM I/O (host pre-layouts everything for contiguous DMA) ----
    qT_d = nc.dram_tensor("qT", [BS, 128, 4, S], bf16, kind="ExternalInput")
    kT_d = nc.dram_tensor("kT", [BS, 128, 4, S], bf16, kind="ExternalInput")
    vT_d = nc.dram_tensor("vT", [BS, 128, 4, S], bf16, kind="ExternalInput")
    Wq_d = nc.dram_tensor("Wq", [128, 4, DK], bf16, kind="ExternalInput")
    Wk_d = nc.dram_tensor("Wk", [128, 4, DK], bf16, kind="ExternalInput")
    Wv_d = nc.dram_tensor("Wv", [128, 4, DK], bf16, kind="ExternalInput")
    bqc_d = nc.dram_tensor("bqc", [DK, 1], f32, kind="ExternalInput")
    bkc_d = nc.dram_tensor("bkc", [DK, 1], f32, kind="ExternalInput")
    bv_d = nc.dram_tensor("bv", [1, DK], bf16, kind="ExternalInput")
    Wo_d = nc.dram_tensor("Wo", [DK, DM], bf16, kind="ExternalInput")
    Wo4_d = nc.dram_tensor("Wo4", [DK, DM], bf16, kind="ExternalInput")
    bo2_d = nc.dram_tensor("bo2", [128, 4], f32, kind="ExternalInput")
    al_d = nc.dram_tensor("alpha", [DK, 1], f32, kind="ExternalInput")
    b4_d = nc.dram_tensor("beta4", [DK, 1], f32, kind="ExternalInput")
    id_d = nc.dram_tensor("ident", [128, 128], bf16, kind="ExternalInput")
    idf_d = nc.dram_tensor("identf", [128, 128], f32, kind="ExternalInput")
    outT_d = nc.dram_tensor("outT", [BS, DM, S], bf16, kind="ExternalOutput")

    class _scope:
        def __init__(self, name):
            self.name = name
        def __enter__(self):
            self.sid, _ = nc.enter_named_scope(self.name, False)
        def __exit__(self, *a):
            nc.leave_named_scope(self.name, self.sid, False)

    with tile.TileContext(nc) as tc:
        with (
            tc.tile_pool(name="persist", bufs=1) as pp,
            tc.tile_pool(name="consts", bufs=1) as cp,
            tc.tile_pool(name="inp", bufs=3) as inp,
            tc.tile_pool(name="work", bufs=6) as wp,
            tc.tile_pool(name="vt", bufs=1) as vtp,
            tc.tile_pool(name="bwork", bufs=3) as bwp,
            tc.tile_pool(name="psum", bufs=1, space="PSUM") as psp,
        ):
            # ---- persistent SBUF ----
            qhT = pp.tile([DK, BS, S], bf16, tag="qhT")
            khT = pp.tile([DK, BS, S], bf16, tag="khT")
            vh_all = pp.tile([128, 4, BS, DK], bf16, tag="vh")   # [j128, jc, c, d]
            rt_all = pp.tile([128, BS, 4, S], bf16, tag="rt")    # [j128, b, jc, i]
            e_all = pp.tile([128, BS, 4, 2 * BS * DK], f32, tag="e")  # [i,b,ic,(u,c,d)]
            heads = pp.tile([DK, BS, S], bf16, tag="heads")       # [d, b, i]
            Z_all = pp.tile([128, 64], f32, tag="Z")   # cols = b*16 + ic*4 + c
            Q_all = pp.tile([128, 64], f32, tag="Q")
            w1_all = pp.tile([128, 64], f32, tag="w1")
            w0_all = pp.tile([128, 16], f32, tag="w0")  # cols = b*4 + ic
            w0T0 = pp.tile([8, 128], f32, tag="w0T0")
            w0T1 = pp.tile([8, 128], f32, tag="w0T1")
            w0f = pp.tile([1, 16 * 128], f32, tag="w0f")

            Wq_s = cp.tile([128, 4, DK], bf16, tag="Wq")
            Wk_s = cp.tile([128, 4, DK], bf16, tag="Wk")
            Wv_s = cp.tile([128, 4, DK], bf16, tag="Wv")
            Wo_s = cp.tile([DK, DM], bf16, tag="Wo")
            Wo4_s = cp.tile([DK, DM], bf16, tag="Wo4")
            bo2_s = cp.tile([128, 4], f32, tag="bo2")
            bqc_s = cp.tile([DK, 1], f32, tag="bqc")
            bkc_s = cp.tile([DK, 1], f32, tag="bkc")
            bv_s = cp.tile([1, DK], bf16, tag="bv")
            al_s = cp.tile([DK, 1], f32, tag="al")
            b4_s = cp.tile([DK, 1], f32, tag="b4")
            id_s = cp.tile([128, 128], bf16, tag="id")
            idf_s = cp.tile([128, 128], f32, tag="idf")
            ones = cp.tile([1, S], bf16, tag="ones")
            ones_f = cp.tile([1, S], f32, tag="ones_f")
            bvb = cp.tile([128, DK], f32, tag="bvb")

            # ---- input k tiles + weights first: they gate the first projs ----
            ktiles = []
            for b in range(BS):
                kt_early = inp.tile([128, 4, S], bf16, tag="kt", name=f"kte{b}")
                nc.sync.dma_start(kt_early[:], kT_d[b])
                ktiles.append(kt_early)
            nc.sync.dma_start(Wk_s[:], Wk_d[:])
            nc.sync.dma_start(Wq_s[:], Wq_d[:])
            nc.sync.dma_start(Wv_s[:], Wv_d[:])
            nc.sync.dma_start(bqc_s[:], bqc_d[:])
            nc.sync.dma_start(bkc_s[:], bkc_d[:])
            nc.sync.dma_start(bv_s[:], bv_d[:])
            nc.sync.dma_start(id_s[:], id_d[:])
            nc.sync.dma_start(idf_s[:], idf_d[:])
            nc.sync.dma_start(Wo_s[:], Wo_d[:])
            nc.sync.dma_start(Wo4_s[:], Wo4_d[:])
            nc.sync.dma_start(bo2_s[:], bo2_d[:])
            nc.sync.dma_start(al_s[:], al_d[:])
            nc.sync.dma_start(b4_s[:], b4_d[:])
            nc.vector.memset(ones[:], 1.0)
            nc.vector.memset(ones_f[:], 1.0)
            nc.scalar.activation(ones_f[0:1, 0:8], ones_f[0:1, 0:8], AF.Exp)
            nc.vector.memset(ones_f[0:1, 0:8], 1.0)

            # Unified PSUM tags:  pe0/pe1: [128,2,512] = 4 banks | su: 2x = 2
            #                     sc: 2x = 2   (total 8 banks)
            def psum_su():
                return psp.tile([128, S], f32, tag="su", bufs=2, name="psu")
            def psum_sc():
                return psp.tile([128, S], f32, tag="sc", bufs=2, name="psc")
            vtiles = []

            # ---- P1: k, q projections ----
            def emit_proj(W_s, b_c, dsrc, tag, dst, b):
                src = inp.tile([128, 4, S], bf16, tag=tag)
                nc.sync.dma_start(src[:], dsrc[b])
                ps = psp.tile([DK, S], f32, tag="su", bufs=2, name="pproj")
                for mc in range(4):
                    nc.tensor.matmul(ps[:], W_s[:, mc, :], src[:, mc, :],
                                     start=(mc == 0), stop=(mc == 3))
                nc.scalar.activation(dst[:, b, :], ps[:], AF.Identity, bias=b_c[:])

            with _scope("P1"):
                for b in range(BS):
                    ps = psp.tile([DK, S], f32, tag="su", bufs=2, name="pprojk")
                    for mc in range(4):
                        nc.tensor.matmul(ps[:], Wk_s[:, mc, :],
                                         ktiles[b][:, mc, :],
                                         start=(mc == 0), stop=(mc == 3))
                    nc.scalar.activation(khT[:, b, :], ps[:], AF.Identity,
                                         bias=bkc_s[:])
                for b in range(BS):
                    emit_proj(Wq_s, bqc_s, qT_d, "qt", qhT, b)
                for b in range(BS):
                    vt = vtp.tile([128, 4, S], bf16, tag=f"vt{b}")
                    nc.sync.dma_start(vt[:], vT_d[b])
                    vtiles.append(vt)

            # ---- P2/P3 machinery ----
            def p2_tail(b, jc, ex):
                su = psum_su()
                for kb in range(4):
                    nc.tensor.matmul(su[:], id_s[:], ex[:, kb, :],
                                     start=(kb == 0), stop=(kb == 3))
                wrec = wp.tile([128, S], f32, tag="wrec")
                nc.vector.reciprocal_approx_fast(wrec[:], su[:])
                eng = nc.gpsimd if jc % 2 == 0 else nc.vector
                eng.tensor_tensor(
                    rt_all[:, b, jc, :], ex[:, b, :], wrec[:], op=OP.mult)

            def emit_p2(b):
                pend = None
                for jc in range(4):
                    ex = wp.tile([128, 4, S], bf16, tag="ex")
                    for half in range(2):
                        ph = psp.tile([128, 2, S], f32, tag=f"pe{half}",
                                      name="ppe")
                        for k2 in range(2):
                            kb = half * 2 + k2
                            nc.tensor.matmul(
                                ph[:, k2, :],
                                khT[:, kb, jc * 128:(jc + 1) * 128],
                                qhT[:, b, :],
                                start=True, stop=True,
                            )
                        nc.scalar.activation(
                            ex[:, half * 2:(half + 1) * 2, :], ph[:], AF.Exp)
                    if pend is not None:
                        p2_tail(*pend)
                    pend = (b, jc, ex)
                p2_tail(*pend)

            def emit_p3(b):
                for ic in range(4):
                    pc = psum_sc()
                    for jc in range(4):
                        nc.tensor.matmul(
                            pc[:, 0:BS * DK],
                            rt_all[:, b, jc, ic * 128:(ic + 1) * 128],
                            vh_all[:, jc].rearrange("p c d -> p (c d)"),
                            start=(jc == 0), stop=(jc == 3),
                        )
                    nc.scalar.activation(e_all[:, b, ic, 0:BS * DK],
                                         pc[:, 0:BS * DK], AF.Exp)
                    nc.scalar.activation(e_all[:, b, ic, BS * DK:2 * BS * DK],
                                         pc[:, 0:BS * DK], AF.Exp, scale=2.0)
                    col = b * 16 + ic * 4
                    zq = wp.tile([128, 2, 4], f32, tag="zq")
                    nc.vector.tensor_reduce(
                        zq[:],
                        e_all[:, b, ic, :].rearrange("p (u g d) -> p u g d", u=2,
                                                     d=DK),
                        axis=AX.X, op=OP.add,
                    )
                    nc.vector.tensor_copy(Z_all[:, col:col + 4], zq[:, 0, :])
                    nc.vector.tensor_copy(Q_all[:, col:col + 4], zq[:, 1, :])

            def emit_vh():
                # bv broadcast
                pb = psum_sc()
                nc.tensor.matmul(pb[:, 0:DK], ones[:, 0:128], bv_s[:],
                                 start=True, stop=True)
                nc.vector.tensor_copy(bvb[:], pb[:, 0:DK])
                for c in range(BS):
                    vt = vtiles[c]
                    for jc in range(4):
                        pv = psum_sc()
                        for mc in range(4):
                            nc.tensor.matmul(
                                pv[:, 0:DK], vt[:, mc, jc * 128:(jc + 1) * 128],
                                Wv_s[:, mc, :],
                                start=(mc == 0), stop=(mc == 3),
                            )
                        nc.vector.tensor_tensor(
                            vh_all[:, jc, c, :], pv[:, 0:DK], bvb[:], op=OP.add)


            # ---- P4: stats per b-pair (overlaps P23 tail) ----
            stp_cm = tc.tile_pool(name="stats", bufs=2)
            stp = stp_cm.__enter__()

            def emit_stats(h):
                c0, c1 = h * 32, (h + 1) * 32
                Zs, Qs = Z_all[:, c0:c1], Q_all[:, c0:c1]
                t = stp.tile([128, 32], f32, tag="t", name="t")
                nc.vector.tensor_tensor(t[:], Zs, Zs, op=OP.mult)
                s = stp.tile([128, 32], f32, tag="s", name="s")
                nc.vector.scalar_tensor_tensor(
                    s[:], t[:], -1.0 / DK, Qs, op0=OP.mult, op1=OP.add)
                rinv = stp.tile([128, 32], f32, tag="rinv", name="rinv")
                nc.vector.reciprocal(rinv[:], t[:])
                v63 = stp.tile([128, 32], f32, tag="v63", name="v63")
                nc.vector.tensor_tensor(v63[:], s[:], rinv[:], op=OP.mult)
                r_ = stp.tile([128, 32], f32, tag="r_", name="r_")
                nc.vector.tensor_scalar(r_[:].bitcast(i32), v63[:].bitcast(i32),
                                        1, None, op0=OP.logical_shift_right)
                nc.vector.tensor_scalar(r_[:].bitcast(i32), r_[:].bitcast(i32),
                                        -1, 0x5F3759DF, op0=OP.mult, op1=OP.add)
                nt = stp.tile([128, 32], f32, tag="nt", name="nt")
                for _ in range(2):
                    nc.vector.tensor_tensor(nt[:], v63[:], r_[:], op=OP.mult)
                    nc.vector.tensor_tensor(nt[:], nt[:], r_[:], op=OP.mult)
                    nc.vector.tensor_scalar(nt[:], nt[:], -0.5, 1.5,
                                            op0=OP.mult, op1=OP.add)
                    nc.vector.tensor_tensor(r_[:], r_[:], nt[:], op=OP.mult)
                R_ = stp.tile([128, 32], f32, tag="R_", name="R_")
                nc.vector.tensor_scalar(R_[:], r_[:], float(np.sqrt(DK - 1.0)),
                                        None, op0=OP.mult)
                u_ = stp.tile([128, 32], f32, tag="u_", name="u_")
                nc.vector.tensor_scalar(u_[:], R_[:], -EPS, 1.0,
                                        op0=OP.mult, op1=OP.add)
                g = stp.tile([128, 32], f32, tag="g", name="g")
                nc.vector.tensor_tensor(g[:], R_[:], u_[:], op=OP.mult)
                zr = stp.tile([128, 32], f32, tag="zr", name="zr")
                nc.vector.reciprocal(zr[:], Zs)
                nc.vector.tensor_tensor(w1_all[:, c0:c1], g[:], zr[:], op=OP.mult)
                gs = stp.tile([128, 8], f32, tag="gs", name="gs")
                nc.vector.tensor_reduce(
                    gs[:], g[:].rearrange("p (s c) -> p s c", c=4), axis=AX.X,
                    op=OP.add)
                nc.vector.tensor_scalar(w0_all[:, h * 8:(h + 1) * 8], gs[:],
                                        -1.0 / DK, None, op0=OP.mult)
                pw = psum_sc()
                nc.tensor.matmul(pw[:8, 0:128], w0_all[:, h * 8:(h + 1) * 8],
                                 idf_s[:], is_transpose=True, start=True,
                                 stop=True)
                w0Th = w0T0 if h == 0 else w0T1
                nc.vector.tensor_copy(w0Th[:, :], pw[:8, 0:128])
                nc.sync.dma_start(
                    w0f[0:1, h * 1024:(h + 1) * 1024]
                    .rearrange("o (s f) -> o s f", s=8),
                    w0Th[:, :])

            # ---- P5 + P6 per-b (P5 DVE ops batched per b) ----
            def emit_p5(b):
                bsc = bwp.tile([128, 4, 4, DK], f32, tag="bsc")  # [i, ic, c, d]
                w1b = (w1_all[:, b * 16:(b + 1) * 16]
                       .rearrange("p (i c) -> p i c", c=4)
                       .unsqueeze(-1).broadcast_to((128, 4, 4, DK)))
                nc.vector.tensor_tensor(
                    bsc[:],
                    e_all[:, b, :, 0:BS * DK].rearrange("p i (c d) -> p i c d",
                                                        d=DK),
                    w1b, op=OP.mult,
                )
                ball = bwp.tile([128, 4, DK], f32, tag="ball")  # [i, ic, d]
                nc.vector.tensor_reduce(
                    ball[:], bsc[:].rearrange("p i c d -> p i d c"),
                    axis=AX.X, op=OP.add,
                )
                # all 4 transposes + w0 rank-1 adds into ONE psum bank
                pbig = psp.tile([DK, S], f32, tag="sc", bufs=2, name="pbig")
                for ic in range(4):
                    nc.tensor.matmul(pbig[:, ic * 128:(ic + 1) * 128],
                                     ball[:, ic, :], idf_s[:],
                                     is_transpose=True, start=True, stop=False)
                    slot = b * 4 + ic
                    nc.tensor.matmul(
                        pbig[:, ic * 128:(ic + 1) * 128], ones_f[:, 0:DK],
                        w0f[0:1, slot * 128:(slot + 1) * 128],
                        start=False, stop=True,
                    )
                nc.vector.tensor_scalar(
                    heads[:, b, :], pbig[:],
                    al_s[:], b4_s[:], op0=OP.mult, op1=OP.add,
                )

            def emit_p6(b):
                for nch in range(4):
                    po = psum_su()
                    nc.tensor.matmul(
                        po[:], Wo_s[:, nch * 128:(nch + 1) * 128],
                        heads[:, b, :], start=True, stop=False,
                    )
                    nc.tensor.matmul(
                        po[:], Wo4_s[:, nch * 128:(nch + 1) * 128],
                        qhT[:, b, :], start=False, stop=True,
                    )
                    ot = bwp.tile([128, S], bf16, tag="ot")
                    nc.scalar.activation(ot[:], po[:], AF.Identity,
                                         bias=bo2_s[:, nch:nch + 1])
                    nc.sync.dma_start(outT_d[b, nch * 128:(nch + 1) * 128, :], ot[:])

            with _scope("P2356"):
                emit_p2(0)
                emit_vh()
                emit_p2(1)
                emit_p3(0)
                emit_p2(2)
                emit_p3(1)
                emit_p2(3)
                emit_p3(2)
                emit_p3(3)
                emit_stats(0)
                emit_stats(1)
                emit_p5(0)
                emit_p5(1)
                emit_p6(0)
                emit_p5(2)
                emit_p6(1)
                emit_p5(3)
                emit_p6(2)
                emit_p6(3)
            stp_cm.__exit__(None, None, None)

    nc._dbg_names = {
        "qhT": qhT.name, "khT": khT.name, "vh_all": vh_all.name,
        "rt_all": rt_all.name, "e_all": e_all.name, "heads": heads.name,
        "Z_all": Z_all.name, "Q_all": Q_all.name, "w1_all": w1_all.name,
        "w0_all": w0_all.name, "w0f": w0f.name,
    }
    return nc


def _build():
    import concourse.bass as bass  # noqa
    import concourse.tile as tile
    from concourse import bacc, mybir

    nc = bacc.Bacc("TRN2", target_bir_lowering=False, debug=False,
                   num_devices=NCORES)
    build_program(nc, tile, mybir)
    nc.compile()
    return nc


_cached_nc = None


def make_in_maps(q, k, v, Wq, bq, Wk, bk, Wv, bv, Wo, bo, alpha, beta):
    import ml_dtypes
    bft = ml_dtypes.bfloat16

    def prelay(x):
        # [S, DM] per batch -> transposed [DM, S] -> [128, 4, S] partition layout
        xT = np.swapaxes(np.asarray(x, np.float32), 1, 2)  # [B, DM, S]
        return np.ascontiguousarray(
            xT.reshape(BS, 4, 128, S).transpose(0, 2, 1, 3)).astype(bft)

    def wlay(W):  # [DM, DK] -> [128, 4, DK]
        return np.ascontiguousarray(
            np.asarray(W, np.float32).reshape(4, 128, DK).transpose(1, 0, 2)
        ).astype(bft)

    qT, kT, vT = prelay(q), prelay(k), prelay(v)
    Wq, Wk, Wv, Wo = (np.asarray(x, np.float32) for x in (Wq, Wk, Wv, Wo))
    bq, bk, bv, bo = (np.asarray(x, np.float32) for x in (bq, bk, bv, bo))
    alpha, beta = np.asarray(alpha, np.float32), np.asarray(beta, np.float32)
    ident = np.eye(128, dtype=ml_dtypes.bfloat16)
    identf = np.eye(128, dtype=np.float32)
    scale = np.float32(1.0 / np.sqrt(np.float32(DK)))  # fenmu sqrt(DK), into Wv
    in_maps = []
    for h in range(NCORES):
        sl = slice(h * DK, (h + 1) * DK)
        in_maps.append({
            "qT": qT, "kT": kT, "vT": vT,
            "Wq": wlay(Wq[:, sl]),
            "Wk": wlay(Wk[:, sl]),
            "Wv": wlay(Wv[:, sl] * scale),
            "bqc": np.ascontiguousarray(bq[sl])[:, None].astype(np.float32),
            "bkc": np.ascontiguousarray(bk[sl])[:, None].astype(np.float32),
            "bv": np.ascontiguousarray(bv[sl] * scale)[None, :].astype(bft),
            "Wo": np.ascontiguousarray(Wo[sl, :]).astype(bft),
            "Wo4": np.ascontiguousarray(4.0 * Wo[sl, :]).astype(bft),
            "bo2": np.ascontiguousarray(
                (bo if h == 0 else np.zeros_like(bo)).reshape(4, 128).T
            ).astype(np.float32),
            "alpha": np.ascontiguousarray(alpha)[:, None],
            "beta4": np.ascontiguousarray(4.0 * beta)[:, None],
            "ident": ident, "identf": identf,
        })
    return in_maps


def assemble(results):
    out = np.zeros((BS, S, DM), np.float32)
    for r in results:
        out += np.swapaxes(np.asarray(r["outT"], np.float32), 1, 2)
    return out


def kernel(**inputs) -> np.ndarray:
    global _cached_nc
    from concourse.bass_utils import run_bass_kernel_spmd

    if _cached_nc is None:
        _cached_nc = _build()
    in_maps = make_in_maps(**inputs)
    res = run_bass_kernel_spmd(_cached_nc, in_maps, list(range(NCORES)))
    return assemble(res.results)



# revision 11
# speedup vs baseline: 1.1144x; 1.1144x over previous
"""Trainium2 Bass kernel for nn_MultiHeadCrossAttention (BS=4, S=512, DM=512, H=8).

Sharding: one attention head per NeuronCore (8 heads / 8 cores). Each core
receives the full (transposed) q/k/v plus its head's weight slices, computes
its head end-to-end including the rank-64 slice of the output projection, and
the host sums the 8 partial outputs.

v2 restructuring vs baseline:
  - q/k projections write DUPLICATED qh/kh into both partition halves
    (stationary hstack(W, W)), enabling 2-way row-packed E-matmuls
    (kb pair in PE rows 0-63 / 64-127 running concurrently).
  - P3 stats via bn_stats/bn_aggr (mean/var of e over d) -> no exp(2x),
    no Z/Q reductions, no reciprocals in stats:
      w1 = (c1/64)*r*u,  r = rsqrt(var),  u = 1 - eps*c1*mean*r,
      w0_col = -mean*w1,  c1 = 8*sqrt(63).
  - P6 uses one matmul per 128-col chunk: stationary vstack(Wo, 4*Wo),
    moving [heads; qh] (heads written to rows 0:64 of the same tile whose
    rows 64:128 hold qh). Output copies on DVE/ACT split.
"""

import numpy as np

BS, S, DM, H, DK = 4, 512, 512, 8, 64
EPS = 1e-6
NCORES = 8
C1 = float(8.0 * np.sqrt(63.0))


def build_program(nc, tile, mybir):
    f32 = mybir.dt.float32
    bf16 = mybir.dt.bfloat16
    i32 = mybir.dt.int32
    AF = mybir.ActivationFunctionType
    OP = mybir.AluOpType
    AX = mybir.AxisListType

    # ---- DRAM I/O (host pre-layouts everything for contiguous DMA) ----
    qT_d = nc.dram_tensor("qT", [BS, 128, 4, S], bf16, kind="ExternalInput")
    kT_d = nc.dram_tensor("kT", [BS, 128, 4, S], bf16, kind="ExternalInput")
    vT_d = nc.dram_tensor("vT", [BS, 128, 4, S], bf16, kind="ExternalInput")
    Wq_d = nc.dram_tensor("Wq2", [128, 4, 128], bf16, kind="ExternalInput")
    Wk_d = nc.dram_tensor("Wk2", [128, 4, 128], bf16, kind="ExternalInput")
    Wv_d = nc.dram_tensor("Wv", [128, 4, DK], bf16, kind="ExternalInput")
    bqc_d = nc.dram_tensor("bqc2", [128, 1], f32, kind="ExternalInput")
    bkc_d = nc.dram_tensor("bkc2", [128, 1], f32, kind="ExternalInput")
    bv_d = nc.dram_tensor("bv", [1, DK], bf16, kind="ExternalInput")
    Wo_d = nc.dram_tensor("WoCat", [128, DM], bf16, kind="ExternalInput")
    bo2_d = nc.dram_tensor("bo2", [128, 4], f32, kind="ExternalInput")
    al_d = nc.dram_tensor("alpha", [DK, 1], f32, kind="ExternalInput")
    b4_d = nc.dram_tensor("beta4", [DK, 1], f32, kind="ExternalInput")
    id_d = nc.dram_tensor("ident", [128, 128], bf16, kind="ExternalInput")
    idf_d = nc.dram_tensor("identf", [128, 128], f32, kind="ExternalInput")
    outT_d = nc.dram_tensor("outT", [BS, DM, S], bf16, kind="ExternalOutput")

    with tile.TileContext(nc) as tc:
        with (
            tc.tile_pool(name="persist", bufs=1) as pp,
            tc.tile_pool(name="consts", bufs=1) as cp,
            tc.tile_pool(name="inp", bufs=3) as inp,
            tc.tile_pool(name="work", bufs=6) as wp,
            tc.tile_pool(name="vt", bufs=1) as vtp,
            tc.tile_pool(name="bwork", bufs=3) as bwp,
            tc.tile_pool(name="psum", bufs=1, space="PSUM") as psp,
        ):
            # ---- persistent SBUF ----
            qhT = pp.tile([128, BS, S], bf16, tag="qhT")   # qh duplicated halves
            khT = pp.tile([128, BS, S], bf16, tag="khT")   # kh duplicated halves
            vh_all = pp.tile([128, 4, BS, DK], bf16, tag="vh")   # [j128, jc, c, d]
            rt_all = pp.tile([128, BS, 4, S], bf16, tag="rt")    # [j128, b, jc, i]
            e_all = pp.tile([128, BS, 4, BS * DK], f32, tag="e")  # [i,b,ic,(c,d)]
            hq = pp.tile([128, BS, S], bf16, tag="hq")  # [0:64]=heads, [64:128]=qh
            # bn_stats raw output: (n, mean, n*var) for even/odd halves of d;
            # one [128, 6] slab per (b, ic, c) — walrus requires exactly 6/part
            bns_all = pp.tile([128, BS, 4, 4, 6], f32, tag="bns")
            w1_all = pp.tile([128, 64], f32, tag="w1")  # cols = b*16 + ic*4 + c
            w0_all = pp.tile([128, 16], f32, tag="w0")  # cols = b*4 + ic
            w0T0 = pp.tile([8, 128], f32, tag="w0T0")
            w0T1 = pp.tile([8, 128], f32, tag="w0T1")
            w0f = pp.tile([1, 16 * 128], f32, tag="w0f")

            Wq_s = cp.tile([128, 4, 128], bf16, tag="Wq")
            Wk_s = cp.tile([128, 4, 128], bf16, tag="Wk")
            Wv_s = cp.tile([128, 4, DK], bf16, tag="Wv")
            Wo_s = cp.tile([128, DM], bf16, tag="Wo")
            bo2_s = cp.tile([128, 4], f32, tag="bo2")
            bqc_s = cp.tile([128, 1], f32, tag="bqc")
            bkc_s = cp.tile([128, 1], f32, tag="bkc")
            bv_s = cp.tile([1, DK], bf16, tag="bv")
            al_s = cp.tile([DK, 1], f32, tag="al")
            b4_s = cp.tile([DK, 1], f32, tag="b4")
            id_s = cp.tile([128, 128], bf16, tag="id")
            idf_s = cp.tile([128, 128], f32, tag="idf")
            ones = cp.tile([1, S], bf16, tag="ones")
            ones_f = cp.tile([1, S], f32, tag="ones_f")
            bvb = cp.tile([128, DK], f32, tag="bvb")

            # ---- input k tiles + weights first: they gate the first projs ----
            ktiles = []
            for b in range(BS):
                kt_early = inp.tile([128, 4, S], bf16, tag="kt", name=f"kte{b}")
                nc.sync.dma_start(kt_early[:], kT_d[b])
                ktiles.append(kt_early)
            nc.sync.dma_start(Wk_s[:], Wk_d[:])
            nc.sync.dma_start(Wq_s[:], Wq_d[:])
            nc.sync.dma_start(Wv_s[:], Wv_d[:])
            nc.sync.dma_start(bqc_s[:], bqc_d[:])
            nc.sync.dma_start(bkc_s[:], bkc_d[:])
            nc.sync.dma_start(bv_s[:], bv_d[:])
            nc.sync.dma_start(id_s[:], id_d[:])
            nc.sync.dma_start(idf_s[:], idf_d[:])
            nc.sync.dma_start(Wo_s[:], Wo_d[:])
            nc.sync.dma_start(bo2_s[:], bo2_d[:])
            nc.sync.dma_start(al_s[:], al_d[:])
            nc.sync.dma_start(b4_s[:], b4_d[:])
            nc.vector.memset(ones[:], 1.0)
            nc.vector.memset(ones_f[:], 1.0)
            nc.scalar.activation(ones_f[0:1, 0:8], ones_f[0:1, 0:8], AF.Exp)
            nc.vector.memset(ones_f[0:1, 0:8], 1.0)

            # PSUM: pe0/pe1 [128,2,512] = 4 banks | su 2x = 2 | sc 2x = 2
            def psum_su():
                return psp.tile([128, S], f32, tag="su", bufs=2, name="psu")
            def psum_sc():
                return psp.tile([128, S], f32, tag="sc", bufs=2, name="psc")
            vtiles = []

            # ---- P1: k, q projections (duplicated-half outputs) ----
            def emit_proj(W_s, b_c, dsrc, tag, dst, b, src=None):
                if src is None:
                    src = inp.tile([128, 4, S], bf16, tag=tag)
                    nc.sync.dma_start(src[:], dsrc[b])
                ps = psp.tile([128, S], f32, tag="su", bufs=2, name="pproj")
                for mc in range(4):
                    nc.tensor.matmul(ps[:], W_s[:, mc, :], src[:, mc, :],
                                     start=(mc == 0), stop=(mc == 3))
                nc.scalar.activation(dst[:, b, :], ps[:], AF.Identity, bias=b_c[:])

            def emit_p1():
                for b in range(BS):
                    emit_proj(Wk_s, bkc_s, kT_d, "kt", khT, b, src=ktiles[b])
                for b in range(BS):
                    emit_proj(Wq_s, bqc_s, qT_d, "qt", qhT, b)
                    # also stash qh into hq rows 64:128 for P6
                for b in range(BS):
                    vt = vtp.tile([128, 4, S], bf16, tag=f"vt{b}")
                    nc.sync.dma_start(vt[:], vT_d[b])
                    vtiles.append(vt)

            def emit_qh_stash(b):
                # copy qh (bf16, duplicated halves) into hq[64:128, b, :]
                nc.vector.tensor_copy(hq[64:128, b, :], qhT[64:128, b, :])

            # ---- P2: E^T matmuls (row-packed pairs) + fenmu + rt ----
            def p2_tail(b, jc, ex):
                su = psum_su()
                for kb in range(4):
                    nc.tensor.matmul(su[:], id_s[:], ex[:, kb, :],
                                     start=(kb == 0), stop=(kb == 3))
                wrec = wp.tile([128, S], f32, tag="wrec")
                nc.vector.reciprocal_approx_fast(wrec[:], su[:])
                eng = nc.gpsimd if jc % 2 == 0 else nc.vector
                eng.tensor_tensor(
                    rt_all[:, b, jc, :], ex[:, b, :], wrec[:], op=OP.mult)

            def emit_p2(b):
                pend = None
                for jc in range(4):
                    ex = wp.tile([128, 4, S], bf16, tag="ex")
                    for pair in range(2):
                        ph = psp.tile([128, 2, S], f32, tag=f"pe{pair}",
                                      name="ppe")
                        nc.tensor.matmul(
                            ph[:, 0, :],
                            khT[0:64, 2 * pair, jc * 128:(jc + 1) * 128],
                            qhT[0:64, b, :], start=True, stop=True)
                        nc.tensor.matmul(
                            ph[:, 1, :],
                            khT[64:128, 2 * pair + 1, jc * 128:(jc + 1) * 128],
                            qhT[64:128, b, :], start=True, stop=True)
                        nc.scalar.activation(
                            ex[:, 2 * pair:2 * pair + 2, :], ph[:], AF.Exp)
                    if pend is not None:
                        p2_tail(*pend)
                    pend = (b, jc, ex)
                p2_tail(*pend)

            # ---- P3: score + exp + bn stats ----
            def emit_p3(b):
                for ic in range(4):
                    pc = psum_sc()
                    for jc in range(4):
                        nc.tensor.matmul(
                            pc[:, 0:BS * DK],
                            rt_all[:, b, jc, ic * 128:(ic + 1) * 128],
                            vh_all[:, jc].rearrange("p c d -> p (c d)"),
                            start=(jc == 0), stop=(jc == 3),
                        )
                    nc.scalar.activation(e_all[:, b, ic, :],
                                         pc[:, 0:BS * DK], AF.Exp)
                    for c in range(BS):
                        nc.vector.bn_stats(
                            bns_all[:, b, ic, c],
                            e_all[:, b, ic, c * DK:(c + 1) * DK])

            def emit_vh():
                pb = psum_sc()
                nc.tensor.matmul(pb[:, 0:DK], ones[:, 0:128], bv_s[:],
                                 start=True, stop=True)
                nc.vector.tensor_copy(bvb[:], pb[:, 0:DK])
                for c in range(BS):
                    vt = vtiles[c]
                    for jc in range(4):
                        pv = psum_sc()
                        for mc in range(4):
                            nc.tensor.matmul(
                                pv[:, 0:DK], vt[:, mc, jc * 128:(jc + 1) * 128],
                                Wv_s[:, mc, :],
                                start=(mc == 0), stop=(mc == 3),
                            )
                        nc.vector.tensor_tensor(
                            vh_all[:, jc, c, :], pv[:, 0:DK], bvb[:], op=OP.add)

            # ---- P4: stats per b-pair (h = b//2): recip-free ----
            stp_cm = tc.tile_pool(name="stats", bufs=2)
            stp = stp_cm.__enter__()

            def emit_stats(h):
                # bn even/odd merge (equal halves of 32): 2m = m_e + m_o,
                # 4*var = (32 v_e + 32 v_o)/16 + (m_e - m_o)^2
                bn = bns_all[:, 2 * h:2 * h + 2]  # [128, 2b, 4ic, 4c, 6]
                m_e, m_o = bn[:, :, :, :, 1], bn[:, :, :, :, 4]
                ve32, vo32 = bn[:, :, :, :, 2], bn[:, :, :, :, 5]
                sa = stp.tile([128, 32], f32, tag="sa", name="sa")
                sa4 = sa[:].rearrange("p (b i c) -> p b i c", b=2, i=4)
                nc.vector.tensor_tensor(sa4, m_e, m_o, op=OP.add)
                de = stp.tile([128, 32], f32, tag="de", name="de")
                de4 = de[:].rearrange("p (b i c) -> p b i c", b=2, i=4)
                nc.vector.tensor_tensor(de4, m_e, m_o, op=OP.subtract)
                sv = stp.tile([128, 32], f32, tag="sv", name="sv")
                sv4 = sv[:].rearrange("p (b i c) -> p b i c", b=2, i=4)
                nc.vector.tensor_tensor(sv4, ve32, vo32, op=OP.add)
                dd = stp.tile([128, 32], f32, tag="dd", name="dd")
                nc.vector.tensor_tensor(dd[:], de[:], de[:], op=OP.mult)
                v4 = stp.tile([128, 32], f32, tag="v4", name="v4")
                nc.vector.scalar_tensor_tensor(
                    v4[:], sv[:], 1.0 / 16.0, dd[:], op0=OP.mult, op1=OP.add)
                # r = rsqrt(v4) bit-trick + 2 Newton; rsqrt(var) = 2r
                r_ = stp.tile([128, 32], f32, tag="r_", name="r_")
                nc.vector.tensor_scalar(r_[:].bitcast(i32), v4[:].bitcast(i32),
                                        1, None, op0=OP.logical_shift_right)
                nc.vector.tensor_scalar(r_[:].bitcast(i32), r_[:].bitcast(i32),
                                        -1, 0x5F3759DF, op0=OP.mult, op1=OP.add)
                nt = stp.tile([128, 32], f32, tag="nt", name="nt")
                for _ in range(2):
                    nc.vector.tensor_tensor(nt[:], v4[:], r_[:], op=OP.mult)
                    nc.vector.tensor_tensor(nt[:], nt[:], r_[:], op=OP.mult)
                    nc.vector.tensor_scalar(nt[:], nt[:], -0.5, 1.5,
                                            op0=OP.mult, op1=OP.add)
                    nc.vector.tensor_tensor(r_[:], r_[:], nt[:], op=OP.mult)
                # u' = sa*r*(-EPS*C1^2/32) + C1/32 ; w1 = r*u' ; w0col = -m*w1
                sar = stp.tile([128, 32], f32, tag="sar", name="sar")
                nc.vector.tensor_tensor(sar[:], sa[:], r_[:], op=OP.mult)
                up = stp.tile([128, 32], f32, tag="up", name="up")
                nc.vector.tensor_scalar(up[:], sar[:], -EPS * C1 * C1 / 32.0,
                                        C1 / 32.0, op0=OP.mult, op1=OP.add)
                c0, c1_ = h * 32, (h + 1) * 32
                w1s = w1_all[:, c0:c1_]
                nc.vector.tensor_tensor(w1s, r_[:], up[:], op=OP.mult)
                w0c = stp.tile([128, 32], f32, tag="w0c", name="w0c")
                nc.vector.scalar_tensor_tensor(
                    w0c[:], sa[:], -0.5, w1s, op0=OP.mult, op1=OP.mult)
                nc.vector.tensor_reduce(
                    w0_all[:, h * 8:(h + 1) * 8],
                    w0c[:].rearrange("p (s c) -> p s c", c=4),
                    axis=AX.X, op=OP.add)
                pw = psum_sc()
                nc.tensor.matmul(pw[:8, 0:128], w0_all[:, h * 8:(h + 1) * 8],
                                 idf_s[:], is_transpose=True, start=True,
                                 stop=True)
                w0Th = w0T0 if h == 0 else w0T1
                nc.vector.tensor_copy(w0Th[:, :], pw[:8, 0:128])
                nc.sync.dma_start(
                    w0f[0:1, h * 1024:(h + 1) * 1024]
                    .rearrange("o (s f) -> o s f", s=8),
                    w0Th[:, :])

            # ---- P5 per-b: weighted c-sum + transpose + LN scale ----
            def emit_p5(b):
                bsc = bwp.tile([128, 4, 4, DK], f32, tag="bsc")  # [i, ic, c, d]
                w1b = (w1_all[:, b * 16:(b + 1) * 16]
                       .rearrange("p (i c) -> p i c", c=4)
                       .unsqueeze(-1).broadcast_to((128, 4, 4, DK)))
                nc.vector.tensor_tensor(
                    bsc[:],
                    e_all[:, b, :, :].rearrange("p i (c d) -> p i c d", d=DK),
                    w1b, op=OP.mult,
                )
                ball = bwp.tile([128, 4, DK], f32, tag="ball")  # [i, ic, d]
                nc.vector.tensor_reduce(
                    ball[:], bsc[:].rearrange("p i c d -> p i d c"),
                    axis=AX.X, op=OP.add,
                )
                pbig = psp.tile([DK, S], f32, tag="sc", bufs=2, name="pbig")
                for ic in range(4):
                    nc.tensor.matmul(pbig[:, ic * 128:(ic + 1) * 128],
                                     ball[:, ic, :], idf_s[:],
                                     is_transpose=True, start=True, stop=False)
                    slot = b * 4 + ic
                    nc.tensor.matmul(
                        pbig[:, ic * 128:(ic + 1) * 128], ones_f[:, 0:DK],
                        w0f[0:1, slot * 128:(slot + 1) * 128],
                        start=False, stop=True,
                    )
                nc.vector.tensor_scalar(
                    hq[0:64, b, :], pbig[:],
                    al_s[:], b4_s[:], op0=OP.mult, op1=OP.add,
                )

            # ---- P6: output projection, one MM per 128-col chunk ----
            def emit_p6(b):
                for nch in range(4):
                    po = psum_su()
                    nc.tensor.matmul(
                        po[:], Wo_s[:, nch * 128:(nch + 1) * 128],
                        hq[:, b, :], start=True, stop=True,
                    )
                    ot = bwp.tile([128, S], bf16, tag="ot")
                    if (b + nch) % 2 == 0:
                        nc.vector.tensor_scalar(
                            ot[:], po[:], 1.0, bo2_s[:, nch:nch + 1],
                            op0=OP.mult, op1=OP.add)
                    else:
                        nc.scalar.activation(ot[:], po[:], AF.Identity,
                                             bias=bo2_s[:, nch:nch + 1])
                    nc.sync.dma_start(outT_d[b, nch * 128:(nch + 1) * 128, :],
                                      ot[:])

            emit_p1()
            emit_p2(0)
            emit_vh()
            for b in range(BS):
                emit_qh_stash(b)
            emit_p2(1)
            emit_p3(0)
            emit_p2(2)
            emit_p3(1)
            emit_p2(3)
            emit_stats(0)
            emit_p3(2)
            emit_p5(0)
            emit_p3(3)
            emit_p5(1)
            emit_p6(0)
            emit_stats(1)
            emit_p6(1)
            emit_p5(2)
            emit_p6(2)
            emit_p5(3)
            emit_p6(3)
            stp_cm.__exit__(None, None, None)

    return nc


def _build():
    import concourse.bass as bass  # noqa
    import concourse.tile as tile
    from concourse import bacc, mybir

    nc = bacc.Bacc("TRN2", target_bir_lowering=False, debug=False,
                   num_devices=NCORES)
    build_program(nc, tile, mybir)
    nc.compile()
    return nc


_cached_nc = None


def make_in_maps(q, k, v, Wq, bq, Wk, bk, Wv, bv, Wo, bo, alpha, beta):
    import ml_dtypes
    bft = ml_dtypes.bfloat16

    def prelay(x):
        # [S, DM] per batch -> transposed [DM, S] -> [128, 4, S] partition layout
        xT = np.swapaxes(np.asarray(x, np.float32), 1, 2)  # [B, DM, S]
        return np.ascontiguousarray(
            xT.reshape(BS, 4, 128, S).transpose(0, 2, 1, 3)).astype(bft)

    def wlay(W):  # [DM, DK] -> [128, 4, DK]
        return np.ascontiguousarray(
            np.asarray(W, np.float32).reshape(4, 128, DK).transpose(1, 0, 2)
        ).astype(bft)

    def wlay2(W):  # [DM, DK] -> hstack duplicate -> [128, 4, 128]
        W2 = np.concatenate([np.asarray(W, np.float32)] * 2, axis=1)
        return np.ascontiguousarray(
            W2.reshape(4, 128, 128).transpose(1, 0, 2)).astype(bft)

    qT, kT, vT = prelay(q), prelay(k), prelay(v)
    Wq, Wk, Wv, Wo = (np.asarray(x, np.float32) for x in (Wq, Wk, Wv, Wo))
    bq, bk, bv, bo = (np.asarray(x, np.float32) for x in (bq, bk, bv, bo))
    alpha, beta = np.asarray(alpha, np.float32), np.asarray(beta, np.float32)
    ident = np.eye(128, dtype=ml_dtypes.bfloat16)
    identf = np.eye(128, dtype=np.float32)
    scale = np.float32(1.0 / np.sqrt(np.float32(DK)))  # fenmu sqrt(DK), into Wv
    in_maps = []
    for h in range(NCORES):
        sl = slice(h * DK, (h + 1) * DK)
        WoCat = np.concatenate([Wo[sl, :], 4.0 * Wo[sl, :]], axis=0)  # [128,DM]
        in_maps.append({
            "qT": qT, "kT": kT, "vT": vT,
            "Wq2": wlay2(Wq[:, sl]),
            "Wk2": wlay2(Wk[:, sl]),
            "Wv": wlay(Wv[:, sl] * scale),
            "bqc2": np.tile(bq[sl], 2)[:, None].astype(np.float32),
            "bkc2": np.tile(bk[sl], 2)[:, None].astype(np.float32),
            "bv": np.ascontiguousarray(bv[sl] * scale)[None, :].astype(bft),
            "WoCat": np.ascontiguousarray(WoCat).astype(bft),
            "bo2": np.ascontiguousarray(
                (bo if h == 0 else np.zeros_like(bo)).reshape(4, 128).T
            ).astype(np.float32),
            "alpha": np.ascontiguousarray(alpha)[:, None],
            "beta4": np.ascontiguousarray(4.0 * beta)[:, None],
            "ident": ident, "identf": identf,
        })
    return in_maps


def assemble(results):
    out = np.zeros((BS, S, DM), np.float32)
    for r in results:
        out += np.swapaxes(np.asarray(r["outT"], np.float32), 1, 2)
    return out


def kernel(**inputs) -> np.ndarray:
    global _cached_nc
    from concourse.bass_utils import run_bass_kernel_spmd

    if _cached_nc is None:
        _cached_nc = _build()
    in_maps = make_in_maps(**inputs)
    res = run_bass_kernel_spmd(_cached_nc, in_maps, list(range(NCORES)))
    return assemble(res.results)


# revision 20
# speedup vs baseline: 1.2381x; 1.1110x over previous
"""Trainium2 Bass kernel for nn_MultiHeadCrossAttention (BS=4, S=512, DM=512, H=8).

Sharding: one attention head per NeuronCore (8 heads / 8 cores). Each core
receives the full (transposed) q/k/v plus its head's weight slices, computes
its head end-to-end including the rank-64 slice of the output projection, and
the host sums the 8 partial outputs.

v2 restructuring vs baseline:
  - q/k projections write DUPLICATED qh/kh into both partition halves
    (stationary hstack(W, W)), enabling 2-way row-packed E-matmuls
    (kb pair in PE rows 0-63 / 64-127 running concurrently).
  - P3 stats via bn_stats/bn_aggr (mean/var of e over d) -> no exp(2x),
    no Z/Q reductions, no reciprocals in stats:
      w1 = (c1/64)*r*u,  r = rsqrt(var),  u = 1 - eps*c1*mean*r,
      w0_col = -mean*w1,  c1 = 8*sqrt(63).
  - P6 uses one matmul per 128-col chunk: stationary vstack(Wo, 4*Wo),
    moving [heads; qh] (heads written to rows 0:64 of the same tile whose
    rows 64:128 hold qh). Output copies on DVE/ACT split.
"""

import numpy as np

BS, S, DM, H, DK = 4, 512, 512, 8, 64
EPS = 1e-6
NCORES = 8
C1 = float(8.0 * np.sqrt(63.0))


def build_program(nc, tile, mybir):
    f32 = mybir.dt.float32
    bf16 = mybir.dt.bfloat16
    i32 = mybir.dt.int32
    AF = mybir.ActivationFunctionType
    OP = mybir.AluOpType
    AX = mybir.AxisListType

    # ---- DRAM I/O (host pre-layouts everything for contiguous DMA) ----
    qT_d = nc.dram_tensor("qT", [BS, 128, 4, S], bf16, kind="ExternalInput")
    kT_d = nc.dram_tensor("kT", [BS, 128, 4, S], bf16, kind="ExternalInput")
    vT_d = nc.dram_tensor("vT", [BS, 128, 4, S], bf16, kind="ExternalInput")
    Wq_d = nc.dram_tensor("Wq2", [128, 4, 128], bf16, kind="ExternalInput")
    Wk_d = nc.dram_tensor("Wk2", [128, 4, 128], bf16, kind="ExternalInput")
    Wv_d = nc.dram_tensor("Wv", [128, 4, DK], bf16, kind="ExternalInput")
    bqc_d = nc.dram_tensor("bqc2", [128, 1], f32, kind="ExternalInput")
    bkc_d = nc.dram_tensor("bkc2", [128, 1], f32, kind="ExternalInput")
    bv_d = nc.dram_tensor("bv", [1, DK], bf16, kind="ExternalInput")
    Wo_d = nc.dram_tensor("WoCat", [128, DM], bf16, kind="ExternalInput")
    bo2_d = nc.dram_tensor("bo2", [128, 4], f32, kind="ExternalInput")
    al_d = nc.dram_tensor("alpha", [DK, 1], f32, kind="ExternalInput")
    b4_d = nc.dram_tensor("beta4", [DK, 1], f32, kind="ExternalInput")
    id_d = nc.dram_tensor("ident", [128, 128], bf16, kind="ExternalInput")
    idf_d = nc.dram_tensor("identf", [128, 128], f32, kind="ExternalInput")
    outT_d = nc.dram_tensor("outT", [BS, DM, S], bf16, kind="ExternalOutput")

    with tile.TileContext(nc) as tc:
        with (
            tc.tile_pool(name="persist", bufs=1) as pp,
            tc.tile_pool(name="consts", bufs=1) as cp,
            tc.tile_pool(name="inp", bufs=3) as inp,
            tc.tile_pool(name="work", bufs=6) as wp,
            tc.tile_pool(name="vt", bufs=1) as vtp,
            tc.tile_pool(name="bwork", bufs=3) as bwp,
            tc.tile_pool(name="psum", bufs=1, space="PSUM") as psp,
        ):
            # ---- persistent SBUF ----
            qhT = pp.tile([128, BS, S], bf16, tag="qhT")   # qh duplicated halves
            khT = pp.tile([128, BS, S], bf16, tag="khT")   # kh duplicated halves
            vh_all = pp.tile([128, 4, BS, DK], bf16, tag="vh")   # [j128, jc, c, d]
            rt_all = pp.tile([128, BS, 4, S], bf16, tag="rt")    # [j128, b, jc, i]
            e_all = pp.tile([128, BS, 4, BS * DK], f32, tag="e")  # [i,b,ic,(c,d)]
            hq = pp.tile([128, BS, S], bf16, tag="hq")  # [0:64]=heads, [64:128]=qh
            # bn_stats raw output: (n, mean, n*var) for even/odd halves of d;
            # one [128, 6] slab per (b, ic, c) — walrus requires exactly 6/part
            bns_all = pp.tile([128, BS, 4, 4, 6], f32, tag="bns")
            w1_all = pp.tile([128, 64], f32, tag="w1")  # cols = b*16 + ic*4 + c
            w0_all = pp.tile([128, 16], f32, tag="w0")  # cols = b*4 + ic

            Wq_s = cp.tile([128, 4, 128], bf16, tag="Wq")
            Wk_s = cp.tile([128, 4, 128], bf16, tag="Wk")
            Wv_s = cp.tile([128, 4, DK], bf16, tag="Wv")
            Wo_s = cp.tile([128, DM], bf16, tag="Wo")
            bo2_s = cp.tile([128, 4], f32, tag="bo2")
            bqc_s = cp.tile([128, 1], f32, tag="bqc")
            bkc_s = cp.tile([128, 1], f32, tag="bkc")
            bv_s = cp.tile([1, DK], bf16, tag="bv")
            al_s = cp.tile([DK, 1], f32, tag="al")
            b4_s = cp.tile([DK, 1], f32, tag="b4")
            id_s = cp.tile([128, 128], bf16, tag="id")
            idf_s = cp.tile([128, 128], f32, tag="idf")
            ones = cp.tile([1, S], bf16, tag="ones")
            ones_f = cp.tile([1, S], f32, tag="ones_f")
            bvb = cp.tile([128, DK], f32, tag="bvb")

            # ---- input DMAs ordered so the b=0 chain unblocks earliest:
            # kt0, kt1, Wk, bkc gate kproj(0,1); qt0, Wq, bqc gate qproj(0);
            # then the rest.
            ktiles = []
            for b in range(2):
                kt_early = inp.tile([128, 4, S], bf16, tag="kt", name=f"kte{b}")
                nc.sync.dma_start(kt_early[:], kT_d[b])
                ktiles.append(kt_early)
            nc.sync.dma_start(Wk_s[:], Wk_d[:])
            nc.sync.dma_start(bkc_s[:], bkc_d[:])
            qt0 = inp.tile([128, 4, S], bf16, tag="qt", name="qte0")
            nc.sync.dma_start(qt0[:], qT_d[0])
            nc.sync.dma_start(Wq_s[:], Wq_d[:])
            nc.sync.dma_start(bqc_s[:], bqc_d[:])
            nc.sync.dma_start(id_s[:], id_d[:])
            for b in range(2, BS):
                kt_early = inp.tile([128, 4, S], bf16, tag="kt", name=f"kte{b}")
                nc.sync.dma_start(kt_early[:], kT_d[b])
                ktiles.append(kt_early)
            vtiles = []
            for c in range(BS):
                vt = vtp.tile([128, 4, S], bf16, tag=f"vt{c}")
                nc.sync.dma_start(vt[:], vT_d[c])
                vtiles.append(vt)
            nc.sync.dma_start(Wv_s[:], Wv_d[:])
            nc.sync.dma_start(bv_s[:], bv_d[:])
            nc.sync.dma_start(idf_s[:], idf_d[:])
            nc.sync.dma_start(Wo_s[:], Wo_d[:])
            nc.sync.dma_start(bo2_s[:], bo2_d[:])
            nc.sync.dma_start(al_s[:], al_d[:])
            nc.sync.dma_start(b4_s[:], b4_d[:])
            nc.vector.memset(ones[:], 1.0)
            nc.vector.memset(ones_f[:], 1.0)
            nc.scalar.activation(ones_f[0:1, 0:8], ones_f[0:1, 0:8], AF.Exp)
            nc.vector.memset(ones_f[0:1, 0:8], 1.0)

            # PSUM: pe0/pe1 [128,2,512] = 4 banks | su 2x = 2 | sc 2x = 2
            def psum_su():
                return psp.tile([128, S], f32, tag="su", bufs=2, name="psu")
            def psum_sc():
                return psp.tile([128, S], f32, tag="sc", bufs=2, name="psc")

            # ---- P1: k, q projections (duplicated-half outputs) ----
            def emit_proj(W_s, b_c, dsrc, tag, dst, b, src=None):
                if src is None:
                    src = inp.tile([128, 4, S], bf16, tag=tag)
                    nc.sync.dma_start(src[:], dsrc[b])
                ps = psp.tile([128, S], f32, tag="su", bufs=2, name="pproj")
                for mc in range(4):
                    nc.tensor.matmul(ps[:], W_s[:, mc, :], src[:, mc, :],
                                     start=(mc == 0), stop=(mc == 3))
                nc.scalar.activation(dst[:, b, :], ps[:], AF.Identity, bias=b_c[:])

            def emit_kproj(b):
                emit_proj(Wk_s, bkc_s, kT_d, "kt", khT, b, src=ktiles[b])

            def emit_qproj(b):
                emit_proj(Wq_s, bqc_s, qT_d, "qt", qhT, b,
                          src=qt0 if b == 0 else None)
                # stash qh (bf16, duplicated halves) into hq rows 64:128 for P6
                nc.vector.tensor_copy(hq[64:128, b, :], qhT[64:128, b, :])

            # ---- P2: E^T matmuls (row-packed pairs) + fenmu + rt ----
            def p2_tail(b, jc, ex):
                su = psum_su()
                for kb in range(4):
                    nc.tensor.matmul(su[:], id_s[:], ex[:, kb, :],
                                     start=(kb == 0), stop=(kb == 3))
                wrec = wp.tile([128, S], f32, tag="wrec")
                nc.vector.reciprocal_approx_fast(wrec[:], su[:])
                nc.gpsimd.tensor_tensor(
                    rt_all[:, b, jc, :], ex[:, b, :], wrec[:], op=OP.mult)

            def emit_p2(b):
                pend = None
                for jc in range(4):
                    ex = wp.tile([128, 4, S], bf16, tag="ex")
                    for pair in range(2):
                        ph = psp.tile([128, 2, S], f32, tag=f"pe{pair}",
                                      name="ppe")
                        nc.tensor.matmul(
                            ph[:, 0, :],
                            khT[0:64, 2 * pair, jc * 128:(jc + 1) * 128],
                            qhT[0:64, b, :], start=True, stop=True)
                        nc.tensor.matmul(
                            ph[:, 1, :],
                            khT[64:128, 2 * pair + 1, jc * 128:(jc + 1) * 128],
                            qhT[64:128, b, :], start=True, stop=True)
                        nc.scalar.activation(
                            ex[:, 2 * pair:2 * pair + 2, :], ph[:], AF.Exp)
                    if pend is not None:
                        p2_tail(*pend)
                    pend = (b, jc, ex)
                p2_tail(*pend)

            # ---- P3: score + exp + bn stats ----
            def emit_p3(b):
                for ic in range(4):
                    pc = psum_sc()
                    for jc in range(4):
                        nc.tensor.matmul(
                            pc[:, 0:BS * DK],
                            rt_all[:, b, jc, ic * 128:(ic + 1) * 128],
                            vh_all[:, jc].rearrange("p c d -> p (c d)"),
                            start=(jc == 0), stop=(jc == 3),
                        )
                    nc.scalar.activation(e_all[:, b, ic, :],
                                         pc[:, 0:BS * DK], AF.Exp)
                    for c in range(BS):
                        nc.vector.bn_stats(
                            bns_all[:, b, ic, c],
                            e_all[:, b, ic, c * DK:(c + 1) * DK])

            def emit_vh():
                pb = psum_sc()
                nc.tensor.matmul(pb[:, 0:DK], ones[:, 0:128], bv_s[:],
                                 start=True, stop=True)
                nc.vector.tensor_copy(bvb[:], pb[:, 0:DK])
                for c in range(BS):
                    vt = vtiles[c]
                    for jc in range(4):
                        pv = psum_sc()
                        for mc in range(4):
                            nc.tensor.matmul(
                                pv[:, 0:DK], vt[:, mc, jc * 128:(jc + 1) * 128],
                                Wv_s[:, mc, :],
                                start=(mc == 0), stop=(mc == 3),
                            )
                        nc.vector.tensor_tensor(
                            vh_all[:, jc, c, :], pv[:, 0:DK], bvb[:], op=OP.add)

            # ---- P4: stats per b-pair (h = b//2): recip-free ----
            stp_cm = tc.tile_pool(name="stats", bufs=2)
            stp = stp_cm.__enter__()

            def emit_stats(h):
                # bn even/odd merge (equal halves of 32): 2m = m_e + m_o,
                # 4*var = (32 v_e + 32 v_o)/16 + (m_e - m_o)^2
                bn = bns_all[:, 2 * h:2 * h + 2]  # [128, 2b, 4ic, 4c, 6]
                m_e, m_o = bn[:, :, :, :, 1], bn[:, :, :, :, 4]
                ve32, vo32 = bn[:, :, :, :, 2], bn[:, :, :, :, 5]
                sa = stp.tile([128, 32], f32, tag="sa", name="sa")
                sa4 = sa[:].rearrange("p (b i c) -> p b i c", b=2, i=4)
                nc.vector.tensor_tensor(sa4, m_e, m_o, op=OP.add)
                de = stp.tile([128, 32], f32, tag="de", name="de")
                de4 = de[:].rearrange("p (b i c) -> p b i c", b=2, i=4)
                nc.vector.tensor_tensor(de4, m_e, m_o, op=OP.subtract)
                sv = stp.tile([128, 32], f32, tag="sv", name="sv")
                sv4 = sv[:].rearrange("p (b i c) -> p b i c", b=2, i=4)
                nc.vector.tensor_tensor(sv4, ve32, vo32, op=OP.add)
                dd = stp.tile([128, 32], f32, tag="dd", name="dd")
                nc.vector.tensor_tensor(dd[:], de[:], de[:], op=OP.mult)
                v4 = stp.tile([128, 32], f32, tag="v4", name="v4")
                nc.vector.scalar_tensor_tensor(
                    v4[:], sv[:], 1.0 / 16.0, dd[:], op0=OP.mult, op1=OP.add)
                # r = rsqrt(v4) bit-trick + 2 Newton; rsqrt(var) = 2r
                r_ = stp.tile([128, 32], f32, tag="r_", name="r_")
                nc.vector.tensor_scalar(r_[:].bitcast(i32), v4[:].bitcast(i32),
                                        1, None, op0=OP.logical_shift_right)
                nc.vector.tensor_scalar(r_[:].bitcast(i32), r_[:].bitcast(i32),
                                        -1, 0x5F3759DF, op0=OP.mult, op1=OP.add)
                nt = stp.tile([128, 32], f32, tag="nt", name="nt")
                for _ in range(2):
                    nc.vector.tensor_tensor(nt[:], v4[:], r_[:], op=OP.mult)
                    nc.vector.tensor_tensor(nt[:], nt[:], r_[:], op=OP.mult)
                    nc.vector.tensor_scalar(nt[:], nt[:], -0.5, 1.5,
                                            op0=OP.mult, op1=OP.add)
                    nc.vector.tensor_tensor(r_[:], r_[:], nt[:], op=OP.mult)
                # u' = sa*r*(-EPS*C1^2/32) + C1/32 ; w1 = r*u' ; w0col = -m*w1
                sar = stp.tile([128, 32], f32, tag="sar", name="sar")
                nc.vector.tensor_tensor(sar[:], sa[:], r_[:], op=OP.mult)
                up = stp.tile([128, 32], f32, tag="up", name="up")
                nc.vector.tensor_scalar(up[:], sar[:], -EPS * C1 * C1 / 32.0,
                                        C1 / 32.0, op0=OP.mult, op1=OP.add)
                c0, c1_ = h * 32, (h + 1) * 32
                w1s = w1_all[:, c0:c1_]
                nc.vector.tensor_tensor(w1s, r_[:], up[:], op=OP.mult)
                w0c = stp.tile([128, 32], f32, tag="w0c", name="w0c")
                nc.vector.scalar_tensor_tensor(
                    w0c[:], sa[:], -0.5, w1s, op0=OP.mult, op1=OP.mult)
                nc.vector.tensor_reduce(
                    w0_all[:, h * 8:(h + 1) * 8],
                    w0c[:].rearrange("p (s c) -> p s c", c=4),
                    axis=AX.X, op=OP.add)

            # ---- P5 per-b: weighted c-sum + transpose + LN scale ----
            def emit_p5(b):
                bsc = bwp.tile([128, 4, 4, DK], f32, tag="bsc")  # [i, ic, c, d]
                w1b = (w1_all[:, b * 16:(b + 1) * 16]
                       .rearrange("p (i c) -> p i c", c=4)
                       .unsqueeze(-1).broadcast_to((128, 4, 4, DK)))
                nc.vector.tensor_tensor(
                    bsc[:],
                    e_all[:, b, :, :].rearrange("p i (c d) -> p i c d", d=DK),
                    w1b, op=OP.mult,
                )
                ball = bwp.tile([128, 4, DK], f32, tag="ball")  # [i, ic, d]
                nc.vector.tensor_reduce(
                    ball[:], bsc[:].rearrange("p i c d -> p i d c"),
                    axis=AX.X, op=OP.add,
                )
                # += w0[b, i] broadcast over d (pre-transpose; kills the
                # w0-transpose + SBUF-reshape-DMA critical path)
                w0b = (w0_all[:, b * 4:(b + 1) * 4]
                       .unsqueeze(-1).broadcast_to((128, 4, DK)))
                nc.vector.tensor_tensor(ball[:], ball[:], w0b, op=OP.add)
                pbig = psp.tile([DK, S], f32, tag="sc", bufs=2, name="pbig")
                for ic in range(4):
                    nc.tensor.matmul(pbig[:, ic * 128:(ic + 1) * 128],
                                     ball[:, ic, :], idf_s[:],
                                     is_transpose=True, start=True, stop=True)
                nc.vector.tensor_scalar(
                    hq[0:64, b, :], pbig[:],
                    al_s[:], b4_s[:], op0=OP.mult, op1=OP.add,
                )

            # ---- P6: output projection, one MM per 128-col chunk ----
            def emit_p6(b):
                for nch in range(4):
                    po = psum_su()
                    nc.tensor.matmul(
                        po[:], Wo_s[:, nch * 128:(nch + 1) * 128],
                        hq[:, b, :], start=True, stop=True,
                    )
                    ot = bwp.tile([128, S], bf16, tag="ot")
                    if (b * 4 + nch) % 4 == 0:
                        nc.vector.tensor_scalar(
                            ot[:], po[:], 1.0, bo2_s[:, nch:nch + 1],
                            op0=OP.mult, op1=OP.add)
                    else:
                        nc.scalar.activation(ot[:], po[:], AF.Identity,
                                             bias=bo2_s[:, nch:nch + 1])
                    nc.sync.dma_start(outT_d[b, nch * 128:(nch + 1) * 128, :],
                                      ot[:])

            emit_kproj(0)
            emit_kproj(1)
            emit_qproj(0)
            emit_kproj(2)
            emit_kproj(3)
            emit_p2(0)
            emit_qproj(1)
            emit_vh()
            emit_p2(1)
            emit_p3(0)
            emit_qproj(2)
            emit_p2(2)
            emit_p3(1)
            emit_qproj(3)
            emit_p2(3)
            emit_stats(0)
            emit_p3(2)
            emit_p5(0)
            emit_p3(3)
            emit_p5(1)
            emit_p6(0)
            emit_stats(1)
            emit_p6(1)
            emit_p5(2)
            emit_p6(2)
            emit_p5(3)
            emit_p6(3)
            stp_cm.__exit__(None, None, None)

    return nc


def _build():
    import concourse.bass as bass  # noqa
    import concourse.tile as tile
    from concourse import bacc, mybir

    nc = bacc.Bacc("TRN2", target_bir_lowering=False, debug=False,
                   num_devices=NCORES)
    build_program(nc, tile, mybir)
    nc.compile()
    return nc


_cached_nc = None


def make_in_maps(q, k, v, Wq, bq, Wk, bk, Wv, bv, Wo, bo, alpha, beta):
    import ml_dtypes
    bft = ml_dtypes.bfloat16

    def prelay(x):
        # [S, DM] per batch -> transposed [DM, S] -> [128, 4, S] partition layout
        xT = np.swapaxes(np.asarray(x, np.float32), 1, 2)  # [B, DM, S]
        return np.ascontiguousarray(
            xT.reshape(BS, 4, 128, S).transpose(0, 2, 1, 3)).astype(bft)

    def wlay(W):  # [DM, DK] -> [128, 4, DK]
        return np.ascontiguousarray(
            np.asarray(W, np.float32).reshape(4, 128, DK).transpose(1, 0, 2)
        ).astype(bft)

    def wlay2(W):  # [DM, DK] -> hstack duplicate -> [128, 4, 128]
        W2 = np.concatenate([np.asarray(W, np.float32)] * 2, axis=1)
        return np.ascontiguousarray(
            W2.reshape(4, 128, 128).transpose(1, 0, 2)).astype(bft)

    qT, kT, vT = prelay(q), prelay(k), prelay(v)
    Wq, Wk, Wv, Wo = (np.asarray(x, np.float32) for x in (Wq, Wk, Wv, Wo))
    bq, bk, bv, bo = (np.asarray(x, np.float32) for x in (bq, bk, bv, bo))
    alpha, beta = np.asarray(alpha, np.float32), np.asarray(beta, np.float32)
    ident = np.eye(128, dtype=ml_dtypes.bfloat16)
    identf = np.eye(128, dtype=np.float32)
    scale = np.float32(1.0 / np.sqrt(np.float32(DK)))  # fenmu sqrt(DK), into Wv
    in_maps = []
    for h in range(NCORES):
        sl = slice(h * DK, (h + 1) * DK)
        WoCat = np.concatenate([Wo[sl, :], 4.0 * Wo[sl, :]], axis=0)  # [128,DM]
        in_maps.append({
            "qT": qT, "kT": kT, "vT": vT,
            "Wq2": wlay2(Wq[:, sl]),
            "Wk2": wlay2(Wk[:, sl]),
            "Wv": wlay(Wv[:, sl] * scale),
            "bqc2": np.tile(bq[sl], 2)[:, None].astype(np.float32),
            "bkc2": np.tile(bk[sl], 2)[:, None].astype(np.float32),
            "bv": np.ascontiguousarray(bv[sl] * scale)[None, :].astype(bft),
            "WoCat": np.ascontiguousarray(WoCat).astype(bft),
            "bo2": np.ascontiguousarray(
                (bo if h == 0 else np.zeros_like(bo)).reshape(4, 128).T
            ).astype(np.float32),
            "alpha": np.ascontiguousarray(alpha)[:, None],
            "beta4": np.ascontiguousarray(4.0 * beta)[:, None],
            "ident": ident, "identf": identf,
        })
    return in_maps


def assemble(results):
    out = np.zeros((BS, S, DM), np.float32)
    for r in results:
        out += np.swapaxes(np.asarray(r["outT"], np.float32), 1, 2)
    return out


def kernel(**inputs) -> np.ndarray:
    global _cached_nc
    from concourse.bass_utils import run_bass_kernel_spmd

    if _cached_nc is None:
        _cached_nc = _build()
    in_maps = make_in_maps(**inputs)
    res = run_bass_kernel_spmd(_cached_nc, in_maps, list(range(NCORES)))
    return assemble(res.results)


# revision 25
# speedup vs baseline: 1.2877x; 1.0400x over previous
"""Trainium2 Bass kernel for nn_MultiHeadCrossAttention (BS=4, S=512, DM=512, H=8).

Sharding: one attention head per NeuronCore (8 heads / 8 cores). Each core
receives the full (transposed) q/k/v plus its head's weight slices, computes
its head end-to-end including the rank-64 slice of the output projection, and
the host sums the 8 partial outputs.

v2 restructuring vs baseline:
  - q/k projections write DUPLICATED qh/kh into both partition halves
    (stationary hstack(W, W)), enabling 2-way row-packed E-matmuls
    (kb pair in PE rows 0-63 / 64-127 running concurrently).
  - P3 stats via bn_stats/bn_aggr (mean/var of e over d) -> no exp(2x),
    no Z/Q reductions, no reciprocals in stats:
      w1 = (c1/64)*r*u,  r = rsqrt(var),  u = 1 - eps*c1*mean*r,
      w0_col = -mean*w1,  c1 = 8*sqrt(63).
  - P6 uses one matmul per 128-col chunk: stationary vstack(Wo, 4*Wo),
    moving [heads; qh] (heads written to rows 0:64 of the same tile whose
    rows 64:128 hold qh). Output copies on DVE/ACT split.
"""

import numpy as np

BS, S, DM, H, DK = 4, 512, 512, 8, 64
EPS = 1e-6
NCORES = 8
C1 = float(8.0 * np.sqrt(63.0))


def build_program(nc, tile, mybir):
    f32 = mybir.dt.float32
    bf16 = mybir.dt.bfloat16
    i32 = mybir.dt.int32
    AF = mybir.ActivationFunctionType
    OP = mybir.AluOpType
    AX = mybir.AxisListType

    # ---- DRAM I/O (host pre-layouts everything for contiguous DMA) ----
    qT_d = nc.dram_tensor("qT", [BS, 128, 4, S], bf16, kind="ExternalInput")
    kT_d = nc.dram_tensor("kT", [BS, 128, 4, S], bf16, kind="ExternalInput")
    vT_d = nc.dram_tensor("vT", [BS, 128, 4, S], bf16, kind="ExternalInput")
    Wq_d = nc.dram_tensor("Wq2", [128, 4, 128], bf16, kind="ExternalInput")
    Wk_d = nc.dram_tensor("Wk2", [128, 4, 128], bf16, kind="ExternalInput")
    Wv_d = nc.dram_tensor("Wv", [128, 4, DK], bf16, kind="ExternalInput")
    bqc_d = nc.dram_tensor("bqc2", [128, 1], f32, kind="ExternalInput")
    bkc_d = nc.dram_tensor("bkc2", [128, 1], f32, kind="ExternalInput")
    bv_d = nc.dram_tensor("bv", [1, DK], bf16, kind="ExternalInput")
    Wo_d = nc.dram_tensor("WoCat", [128, DM], bf16, kind="ExternalInput")
    bo2_d = nc.dram_tensor("bo2", [128, 4], f32, kind="ExternalInput")
    al_d = nc.dram_tensor("alpha", [DK, 1], f32, kind="ExternalInput")
    b4_d = nc.dram_tensor("beta4", [DK, 1], f32, kind="ExternalInput")
    id_d = nc.dram_tensor("ident", [128, 128], bf16, kind="ExternalInput")
    idf_d = nc.dram_tensor("identf", [128, 128], f32, kind="ExternalInput")
    outT_d = nc.dram_tensor("outT", [BS, DM, S], bf16, kind="ExternalOutput")

    with tile.TileContext(nc) as tc:
        with (
            tc.tile_pool(name="persist", bufs=1) as pp,
            tc.tile_pool(name="consts", bufs=1) as cp,
            tc.tile_pool(name="inp", bufs=3) as inp,
            tc.tile_pool(name="work", bufs=6) as wp,
            tc.tile_pool(name="vt", bufs=1) as vtp,
            tc.tile_pool(name="bwork", bufs=3) as bwp,
            tc.tile_pool(name="psum", bufs=1, space="PSUM") as psp,
        ):
            # ---- persistent SBUF ----
            qhT = pp.tile([128, BS, S], bf16, tag="qhT")   # qh duplicated halves
            khT = pp.tile([128, BS, S], bf16, tag="khT")   # kh duplicated halves
            vh_all = pp.tile([128, 4, BS, DK], bf16, tag="vh")   # [j128, jc, c, d]
            rt_all = pp.tile([128, BS, 4, S], bf16, tag="rt")    # [j128, b, jc, i]
            e_all = pp.tile([128, BS, 4, BS * DK], f32, tag="e")  # [i,b,ic,(c,d)]
            hq = pp.tile([128, BS, S], bf16, tag="hq")  # [0:64]=heads, [64:128]=qh
            # bn_stats raw output: (n, mean, n*var) for even/odd halves of d;
            # one [128, 6] slab per (b, ic, c) — walrus requires exactly 6/part
            bns_all = pp.tile([128, BS, 4, 4, 6], f32, tag="bns")
            w1_all = pp.tile([128, 64], f32, tag="w1")  # cols = b*16 + ic*4 + c
            w0_all = pp.tile([128, 16], f32, tag="w0")  # cols = b*4 + ic

            Wq_s = cp.tile([128, 4, 128], bf16, tag="Wq")
            Wk_s = cp.tile([128, 4, 128], bf16, tag="Wk")
            Wv_s = cp.tile([128, 4, DK], bf16, tag="Wv")
            Wo_s = cp.tile([128, DM], bf16, tag="Wo")
            bo2_s = cp.tile([128, 4], f32, tag="bo2")
            bqc_s = cp.tile([128, 1], f32, tag="bqc")
            bkc_s = cp.tile([128, 1], f32, tag="bkc")
            bv_s = cp.tile([1, DK], bf16, tag="bv")
            al_s = cp.tile([DK, 1], f32, tag="al")
            b4_s = cp.tile([DK, 1], f32, tag="b4")
            id_s = cp.tile([128, 128], bf16, tag="id")
            idf_s = cp.tile([128, 128], f32, tag="idf")
            ones = cp.tile([1, S], bf16, tag="ones")
            ones_f = cp.tile([1, S], f32, tag="ones_f")
            bvb = cp.tile([128, DK], f32, tag="bvb")

            # ---- input DMAs ordered so the b=0 chain unblocks earliest:
            # kt0, kt1, Wk, bkc gate kproj(0,1); qt0, Wq, bqc gate qproj(0);
            # then the rest.
            ktiles = []
            for b in range(2):
                kt_early = inp.tile([128, 4, S], bf16, tag="kt", name=f"kte{b}")
                nc.sync.dma_start(kt_early[:], kT_d[b])
                ktiles.append(kt_early)
            nc.sync.dma_start(Wk_s[:], Wk_d[:])
            nc.sync.dma_start(bkc_s[:], bkc_d[:])
            qt0 = inp.tile([128, 4, S], bf16, tag="qt", name="qte0")
            nc.sync.dma_start(qt0[:], qT_d[0])
            nc.sync.dma_start(Wq_s[:], Wq_d[:])
            nc.sync.dma_start(bqc_s[:], bqc_d[:])
            nc.sync.dma_start(id_s[:], id_d[:])
            for b in range(2, BS):
                kt_early = inp.tile([128, 4, S], bf16, tag="kt", name=f"kte{b}")
                nc.sync.dma_start(kt_early[:], kT_d[b])
                ktiles.append(kt_early)
            vtiles = []
            for c in range(BS):
                vt = vtp.tile([128, 4, S], bf16, tag=f"vt{c}")
                nc.sync.dma_start(vt[:], vT_d[c])
                vtiles.append(vt)
            nc.sync.dma_start(Wv_s[:], Wv_d[:])
            nc.sync.dma_start(bv_s[:], bv_d[:])
            nc.sync.dma_start(idf_s[:], idf_d[:])
            nc.sync.dma_start(Wo_s[:], Wo_d[:])
            nc.sync.dma_start(bo2_s[:], bo2_d[:])
            nc.sync.dma_start(al_s[:], al_d[:])
            nc.sync.dma_start(b4_s[:], b4_d[:])
            nc.vector.memset(ones[:], 1.0)
            nc.vector.memset(ones_f[:], 1.0)
            nc.scalar.activation(ones_f[0:1, 0:8], ones_f[0:1, 0:8], AF.Exp)
            nc.vector.memset(ones_f[0:1, 0:8], 1.0)

            # PSUM: pe0/pe1 [128,2,512] = 4 banks | su 2x = 2 | sc 2x = 2
            def psum_su():
                return psp.tile([128, S], f32, tag="su", bufs=2, name="psu")
            def psum_sc():
                return psp.tile([128, S], f32, tag="sc", bufs=2, name="psc")

            # ---- P1: k, q projections (duplicated-half outputs) ----
            def emit_proj(W_s, b_c, dsrc, tag, dst, b, src=None, dve=False):
                if src is None:
                    src = inp.tile([128, 4, S], bf16, tag=tag)
                    nc.sync.dma_start(src[:], dsrc[b])
                ps = psp.tile([128, S], f32, tag="su", bufs=2, name="pproj")
                for mc in range(4):
                    nc.tensor.matmul(ps[:], W_s[:, mc, :], src[:, mc, :],
                                     start=(mc == 0), stop=(mc == 3))
                if dve:
                    nc.vector.tensor_scalar(dst[:, b, :], ps[:], 1.0, b_c[:],
                                            op0=OP.mult, op1=OP.add)
                else:
                    nc.scalar.activation(dst[:, b, :], ps[:], AF.Identity,
                                         bias=b_c[:])

            def emit_kproj(b):
                emit_proj(Wk_s, bkc_s, kT_d, "kt", khT, b, src=ktiles[b])

            def emit_qproj(b):
                emit_proj(Wq_s, bqc_s, qT_d, "qt", qhT, b,
                          src=qt0 if b == 0 else None, dve=True)
                # stash qh (bf16, duplicated halves) into hq rows 64:128 for P6
                nc.vector.tensor_copy(hq[64:128, b, :], qhT[64:128, b, :])

            # ---- P2: E^T matmuls (row-packed pairs) + fenmu + rt ----
            def p2_tail(b, jc, ex):
                su = psum_su()
                for kb in range(4):
                    nc.tensor.matmul(su[:], id_s[:], ex[:, kb, :],
                                     start=(kb == 0), stop=(kb == 3))
                wrec = wp.tile([128, S], f32, tag="wrec")
                nc.vector.reciprocal_approx_fast(wrec[:], su[:])
                nc.gpsimd.tensor_tensor(
                    rt_all[:, b, jc, :], ex[:, b, :], wrec[:], op=OP.mult)

            def emit_p2(b):
                pend = None
                for jc in range(4):
                    ex = wp.tile([128, 4, S], bf16, tag="ex")
                    for pair in range(2):
                        ph = psp.tile([128, 2, S], f32, tag=f"pe{pair}",
                                      name="ppe")
                        nc.tensor.matmul(
                            ph[:, 0, :],
                            khT[0:64, 2 * pair, jc * 128:(jc + 1) * 128],
                            qhT[0:64, b, :], start=True, stop=True)
                        nc.tensor.matmul(
                            ph[:, 1, :],
                            khT[64:128, 2 * pair + 1, jc * 128:(jc + 1) * 128],
                            qhT[64:128, b, :], start=True, stop=True)
                        nc.scalar.activation(
                            ex[:, 2 * pair:2 * pair + 2, :], ph[:], AF.Exp)
                    if pend is not None:
                        p2_tail(*pend)
                    pend = (b, jc, ex)
                p2_tail(*pend)

            # ---- P3: score + exp + bn stats ----
            def emit_p3(b):
                for ic in range(4):
                    pc = psum_sc()
                    for jc in range(4):
                        nc.tensor.matmul(
                            pc[:, 0:BS * DK],
                            rt_all[:, b, jc, ic * 128:(ic + 1) * 128],
                            vh_all[:, jc].rearrange("p c d -> p (c d)"),
                            start=(jc == 0), stop=(jc == 3),
                        )
                    nc.scalar.activation(e_all[:, b, ic, :],
                                         pc[:, 0:BS * DK], AF.Exp)
                    for c in range(BS):
                        nc.vector.bn_stats(
                            bns_all[:, b, ic, c],
                            e_all[:, b, ic, c * DK:(c + 1) * DK])

            def emit_vh():
                pb = psum_sc()
                nc.tensor.matmul(pb[:, 0:DK], ones[:, 0:128], bv_s[:],
                                 start=True, stop=True)
                nc.vector.tensor_copy(bvb[:], pb[:, 0:DK])
                for c in range(BS):
                    vt = vtiles[c]
                    for jc in range(4):
                        pv = psum_sc()
                        for mc in range(4):
                            nc.tensor.matmul(
                                pv[:, 0:DK], vt[:, mc, jc * 128:(jc + 1) * 128],
                                Wv_s[:, mc, :],
                                start=(mc == 0), stop=(mc == 3),
                            )
                        nc.vector.tensor_tensor(
                            vh_all[:, jc, c, :], pv[:, 0:DK], bvb[:], op=OP.add)

            # ---- P4: stats per b-pair (h = b//2): recip-free ----
            stp_cm = tc.tile_pool(name="stats", bufs=2)
            stp = stp_cm.__enter__()

            def emit_stats(b):
                # bn even/odd merge (equal halves of 32): 2m = m_e + m_o,
                # 4*var = (32 v_e + 32 v_o)/16 + (m_e - m_o)^2
                bn = bns_all[:, b]  # [128, 4ic, 4c, 6]
                m_e, m_o = bn[:, :, :, 1], bn[:, :, :, 4]
                ve32, vo32 = bn[:, :, :, 2], bn[:, :, :, 5]
                sa = stp.tile([128, 16], f32, tag="sa", name="sa")
                sa4 = sa[:].rearrange("p (i c) -> p i c", i=4)
                nc.vector.tensor_tensor(sa4, m_e, m_o, op=OP.add)
                de = stp.tile([128, 16], f32, tag="de", name="de")
                de4 = de[:].rearrange("p (i c) -> p i c", i=4)
                nc.vector.tensor_tensor(de4, m_e, m_o, op=OP.subtract)
                sv = stp.tile([128, 16], f32, tag="sv", name="sv")
                sv4 = sv[:].rearrange("p (i c) -> p i c", i=4)
                nc.vector.tensor_tensor(sv4, ve32, vo32, op=OP.add)
                dd = stp.tile([128, 16], f32, tag="dd", name="dd")
                nc.vector.tensor_tensor(dd[:], de[:], de[:], op=OP.mult)
                v4 = stp.tile([128, 16], f32, tag="v4", name="v4")
                nc.vector.scalar_tensor_tensor(
                    v4[:], sv[:], 1.0 / 16.0, dd[:], op0=OP.mult, op1=OP.add)
                # r = rsqrt(v4) bit-trick + 2 Newton; rsqrt(var) = 2r
                r_ = stp.tile([128, 16], f32, tag="r_", name="r_")
                nc.vector.tensor_scalar(r_[:].bitcast(i32), v4[:].bitcast(i32),
                                        1, None, op0=OP.logical_shift_right)
                nc.vector.tensor_scalar(r_[:].bitcast(i32), r_[:].bitcast(i32),
                                        -1, 0x5F3759DF, op0=OP.mult, op1=OP.add)
                nt = stp.tile([128, 16], f32, tag="nt", name="nt")
                for _ in range(2):
                    nc.vector.tensor_tensor(nt[:], v4[:], r_[:], op=OP.mult)
                    nc.vector.tensor_tensor(nt[:], nt[:], r_[:], op=OP.mult)
                    nc.vector.tensor_scalar(nt[:], nt[:], -0.5, 1.5,
                                            op0=OP.mult, op1=OP.add)
                    nc.vector.tensor_tensor(r_[:], r_[:], nt[:], op=OP.mult)
                # u' = sa*r*(-EPS*C1^2/32) + C1/32 ; w1 = r*u' ; w0col = -m*w1
                sar = stp.tile([128, 16], f32, tag="sar", name="sar")
                nc.vector.tensor_tensor(sar[:], sa[:], r_[:], op=OP.mult)
                up = stp.tile([128, 16], f32, tag="up", name="up")
                nc.vector.tensor_scalar(up[:], sar[:], -EPS * C1 * C1 / 32.0,
                                        C1 / 32.0, op0=OP.mult, op1=OP.add)
                w1s = w1_all[:, b * 16:(b + 1) * 16]
                nc.vector.tensor_tensor(w1s, r_[:], up[:], op=OP.mult)
                w0c = stp.tile([128, 16], f32, tag="w0c", name="w0c")
                nc.vector.scalar_tensor_tensor(
                    w0c[:], sa[:], -0.5, w1s, op0=OP.mult, op1=OP.mult)
                nc.vector.tensor_reduce(
                    w0_all[:, b * 4:(b + 1) * 4],
                    w0c[:].rearrange("p (s c) -> p s c", c=4),
                    axis=AX.X, op=OP.add)

            # ---- P5 per-b: weighted c-sum + transpose + LN scale ----
            def emit_p5(b):
                bsc = bwp.tile([128, 4, 4, DK], f32, tag="bsc")  # [i, ic, c, d]
                w1b = (w1_all[:, b * 16:(b + 1) * 16]
                       .rearrange("p (i c) -> p i c", c=4)
                       .unsqueeze(-1).broadcast_to((128, 4, 4, DK)))
                nc.vector.tensor_tensor(
                    bsc[:],
                    e_all[:, b, :, :].rearrange("p i (c d) -> p i c d", d=DK),
                    w1b, op=OP.mult,
                )
                ball = bwp.tile([128, 4, DK], f32, tag="ball")  # [i, ic, d]
                nc.vector.tensor_reduce(
                    ball[:], bsc[:].rearrange("p i c d -> p i d c"),
                    axis=AX.X, op=OP.add,
                )
                # += w0[b, i] broadcast over d (pre-transpose; kills the
                # w0-transpose + SBUF-reshape-DMA critical path)
                w0b = (w0_all[:, b * 4:(b + 1) * 4]
                       .unsqueeze(-1).broadcast_to((128, 4, DK)))
                nc.vector.tensor_tensor(ball[:], ball[:], w0b, op=OP.add)
                pbig = psp.tile([DK, S], f32, tag="sc", bufs=2, name="pbig")
                for ic in range(4):
                    nc.tensor.matmul(pbig[:, ic * 128:(ic + 1) * 128],
                                     ball[:, ic, :], idf_s[:],
                                     is_transpose=True, start=True, stop=True)
                nc.scalar.activation(hq[0:64, b, :], pbig[:], AF.Identity,
                                     bias=b4_s[:], scale=al_s[:])

            # ---- P6: output projection, one MM per 128-col chunk ----
            def emit_p6(b):
                for nch in range(4):
                    po = psum_su()
                    nc.tensor.matmul(
                        po[:], Wo_s[:, nch * 128:(nch + 1) * 128],
                        hq[:, b, :], start=True, stop=True,
                    )
                    ot = bwp.tile([128, S], bf16, tag="ot")
                    if (b * 4 + nch) % 4 == 0:
                        nc.vector.tensor_scalar(
                            ot[:], po[:], 1.0, bo2_s[:, nch:nch + 1],
                            op0=OP.mult, op1=OP.add)
                    else:
                        nc.scalar.activation(ot[:], po[:], AF.Identity,
                                             bias=bo2_s[:, nch:nch + 1])
                    nc.sync.dma_start(outT_d[b, nch * 128:(nch + 1) * 128, :],
                                      ot[:])

            emit_kproj(0)
            emit_kproj(1)
            emit_qproj(0)
            emit_kproj(2)
            emit_kproj(3)
            emit_p2(0)
            emit_qproj(1)
            emit_vh()
            emit_p2(1)
            emit_p3(0)
            emit_qproj(2)
            emit_p2(2)
            emit_p3(1)
            emit_stats(0)
            emit_qproj(3)
            emit_p2(3)
            emit_p3(2)
            emit_stats(1)
            emit_p5(0)
            emit_p6(0)
            emit_p3(3)
            emit_stats(2)
            emit_p5(1)
            emit_p6(1)
            emit_stats(3)
            emit_p5(2)
            emit_p6(2)
            emit_p5(3)
            emit_p6(3)
            stp_cm.__exit__(None, None, None)

    return nc


def _build():
    import concourse.bass as bass  # noqa
    import concourse.tile as tile
    from concourse import bacc, mybir

    nc = bacc.Bacc("TRN2", target_bir_lowering=False, debug=False,
                   num_devices=NCORES)
    build_program(nc, tile, mybir)
    nc.compile()
    return nc


_cached_nc = None


def make_in_maps(q, k, v, Wq, bq, Wk, bk, Wv, bv, Wo, bo, alpha, beta):
    import ml_dtypes
    bft = ml_dtypes.bfloat16

    def prelay(x):
        # [S, DM] per batch -> transposed [DM, S] -> [128, 4, S] partition layout
        xT = np.swapaxes(np.asarray(x, np.float32), 1, 2)  # [B, DM, S]
        return np.ascontiguousarray(
            xT.reshape(BS, 4, 128, S).transpose(0, 2, 1, 3)).astype(bft)

    def wlay(W):  # [DM, DK] -> [128, 4, DK]
        return np.ascontiguousarray(
            np.asarray(W, np.float32).reshape(4, 128, DK).transpose(1, 0, 2)
        ).astype(bft)

    def wlay2(W):  # [DM, DK] -> hstack duplicate -> [128, 4, 128]
        W2 = np.concatenate([np.asarray(W, np.float32)] * 2, axis=1)
        return np.ascontiguousarray(
            W2.reshape(4, 128, 128).transpose(1, 0, 2)).astype(bft)

    qT, kT, vT = prelay(q), prelay(k), prelay(v)
    Wq, Wk, Wv, Wo = (np.asarray(x, np.float32) for x in (Wq, Wk, Wv, Wo))
    bq, bk, bv, bo = (np.asarray(x, np.float32) for x in (bq, bk, bv, bo))
    alpha, beta = np.asarray(alpha, np.float32), np.asarray(beta, np.float32)
    ident = np.eye(128, dtype=ml_dtypes.bfloat16)
    identf = np.eye(128, dtype=np.float32)
    scale = np.float32(1.0 / np.sqrt(np.float32(DK)))  # fenmu sqrt(DK), into Wv
    in_maps = []
    for h in range(NCORES):
        sl = slice(h * DK, (h + 1) * DK)
        WoCat = np.concatenate([Wo[sl, :], 4.0 * Wo[sl, :]], axis=0)  # [128,DM]
        in_maps.append({
            "qT": qT, "kT": kT, "vT": vT,
            "Wq2": wlay2(Wq[:, sl]),
            "Wk2": wlay2(Wk[:, sl]),
            "Wv": wlay(Wv[:, sl] * scale),
            "bqc2": np.tile(bq[sl], 2)[:, None].astype(np.float32),
            "bkc2": np.tile(bk[sl], 2)[:, None].astype(np.float32),
            "bv": np.ascontiguousarray(bv[sl] * scale)[None, :].astype(bft),
            "WoCat": np.ascontiguousarray(WoCat).astype(bft),
            "bo2": np.ascontiguousarray(
                (bo if h == 0 else np.zeros_like(bo)).reshape(4, 128).T
            ).astype(np.float32),
            "alpha": np.ascontiguousarray(alpha)[:, None],
            "beta4": np.ascontiguousarray(4.0 * beta)[:, None],
            "ident": ident, "identf": identf,
        })
    return in_maps


def assemble(results):
    out = np.zeros((BS, S, DM), np.float32)
    for r in results:
        out += np.swapaxes(np.asarray(r["outT"], np.float32), 1, 2)
    return out


def kernel(**inputs) -> np.ndarray:
    global _cached_nc
    from concourse.bass_utils import run_bass_kernel_spmd

    if _cached_nc is None:
        _cached_nc = _build()
    in_maps = make_in_maps(**inputs)
    res = run_bass_kernel_spmd(_cached_nc, in_maps, list(range(NCORES)))
    return assemble(res.results)
